# revision 4
# baseline (speedup 1.0000x reference)
import numpy as np

# nn_AttentionPooling: pooled = segsum(softmax_seg(MLP(x)) * x) @ Wp + bp
# N=1M nodes, D=256, B=4096 segments, batch sorted. 8 NeuronCores.
#
# Strategy: shard nodes at segment boundaries so core c owns segments
# [512c, 512(c+1)) exactly -> the segment reduction is fully core-local and
# no collective is needed. Within a core, nodes are further split at every
# 128-segment boundary into 4 "groups"; each group accumulates a PSUM chunk
# U[128 segs, 256+1] via one-hot weighted matmuls (one-hot built on-device
# from host-precomputed relative segment ids). exp(s) is computed with a
# fixed offset C instead of the per-segment max (mathematically identical
# softmax; s is bounded by ||w2||_1 so no overflow).
#
# v4: the per-exec dispatch overhead scales with the number of input
# tensors (~36us each) and with the effectful (slow-path) jax dispatch.
# All inputs are packed into few tensors and bass_fast_dispatch is enabled
# so jit execs take the C++ fast path. The shipped natural layout carries
# an interleaved ones column per subtile so the denominator rides the
# pooling matmul (1 matmul per subtile).
# v5: the score MLP's first layer runs as fp8e4m3 DoubleRow matmuls
# (K=256 contracted in one pass at 2 rows/cycle): one matmul per dout
# block per super instead of four, ~4x less PE time. W1 ships x8 in fp8
# (avoids subnormals), rescaled by the tanh activation's scale=1/8.
# The transposed x layout ships as fp8 (score path only -> ~0.5% score
# noise, harmless); the natural layout for pooling stays fp16, shipped
# for ALL supers (no on-device transposes).

N = 1_000_000
D = 256
B = 4096
NCORES = 8
SEGS_PER_CORE = B // NCORES          # 512
CHUNK = 128                          # segments per PSUM chunk
GROUPS = SEGS_PER_CORE // CHUNK      # 4
SUB = 128                            # nodes per subtile (partition dim)
SPS = 4                              # subtiles per super-tile
C_OFF = 0.0                          # exp(s - C_OFF); s+b2 in [-5,5] fits fp16/f32
W1SCALE = 8.0                        # W1 shipped x8 in fp8, undone by ACT scale
WCH = SPS * 257                      # data chunk width (1028)

_patched = False
WORK_FRAC = 1.0  # debug knob: fraction of super-tiles emitted (timing experiments)


def _patch_drain():
    """walrus core_v3 allows 1 sync-wait per CTRL drain; split Tile's tail
    drain waits across a chain of drains."""
    global _patched
    if _patched:
        return
    import concourse.tile as tile_mod

    def _split_drain_and_barrier(self, tick_clock, wait_clock):
        drain_inst = self.nc.sync.drain()
        wait_clock.add_sem_waits(
            drain_inst.ins, tile_mod.ScopedClock({None: tick_clock.global_clock})
        )
        si = drain_inst.ins.sync_info
        if si is not None and si.on_wait is not None and len(si.on_wait) > 1:
            waits = list(si.on_wait)
            SI = type(si)
            si.on_wait = waits[:1]
            for w in waits[1:]:
                extra = self.nc.sync.drain()
                extra.ins.sync_info = SI(on_wait=[w], on_update=[])
        self.nc.all_engine_barrier()
        assert self.sems is not None
        popped = self.nc._tile_sem_poison_stack.pop()
        assert popped is self._sem_poison
        self.nc.clear_and_free_semaphores(list(self.sems.allocated().values()))
        self.nc.all_engine_barrier()

    tile_mod.TileContext._drain_and_barrier = _split_drain_and_barrier

    # Split >1-wait instructions: walrus codegen has tiny per-instruction
    # sync-wait caps. Insert same-engine NOPs carrying the excess waits.
    import concourse.mybir as mybir
    _orig_lower = tile_mod.TileContext._lower_ordered_insts

    def _lower_with_wait_split(self, ordered):
        for bbname in list(ordered.keys()):
            insts = ordered[bbname]
            newl = []
            for inst in insts:
                si = getattr(inst, "sync_info", None)
                eng = getattr(inst, "engine", None)
                ow = list(si.on_wait) if (si is not None and si.on_wait) else []
                if (
                    len(ow) > 1
                    and eng is not None
                    and eng in self.nc.engines
                    and not isinstance(inst, tile_mod.TileBranchInst)
                ):
                    SI = type(si)
                    si.on_wait = ow[-1:]
                    for w in ow[:-1]:
                        nop = self.nc.engines[eng].nop(nofuse=True, hint="wsplit")
                        nop.ins.sync_info = SI(on_wait=[w], on_update=[])
                        newl.append(nop.ins)
                newl.append(inst)
            ordered[bbname] = newl
        return _orig_lower(self, ordered)

    tile_mod.TileContext._lower_ordered_insts = _lower_with_wait_split
    _patched = True


def _fast_dispatch():
    """Enable concourse's C++ fast-path jit dispatch (drops the bass_exec
    effect token). Roughly halves the per-exec python dispatch overhead on
    the axon client; computation semantics are unchanged."""
    try:
        import jax
        import concourse.bass2jax  # noqa: F401  (registers the config)
        jax.config.update("bass_fast_dispatch", True)
    except Exception:
        pass


# consts16 column offsets (inside the last f16 data chunk)
C16_W2, C16_IOTA, C16_ONE = 0, 2, 130
C16_END = 131
# consts32 column offsets
C32_WP, C32_BP, C32_B1, C32_EB, C32_IDF, C32_REL = 0, 512, 768, 770, 771, 899


def _build_nc(n_super_per_group):
    import concourse.bass as bass
    import concourse.mybir as mybir
    from concourse.tile import TileContext

    dt = mybir.dt
    f32 = dt.float32
    f16 = dt.float16
    f8 = dt.float8e4
    Alu = mybir.AluOpType
    Act = mybir.ActivationFunctionType
    DR = mybir.MatmulPerfMode.DoubleRow

    SG = n_super_per_group
    n_super = GROUPS * SG
    n_sub = n_super * SPS

    nc = bass.Bass(target_bir_lowering=False, use_seq_codegen=True)

    # f16 blob: natural-x chunks (ones col interleaved) + small f16 consts
    data = nc.declare_dram_parameter("data", [n_super + 1, SUB, WCH], f16, isOutput=False)
    # fp8 blob: DoubleRow-layout xT chunks + W1 chunk
    data8 = nc.declare_dram_parameter("data8", [n_super + 1, SUB, 2, 512], f8, isOutput=False)
    c32in = nc.declare_dram_parameter("c32", [128, C32_REL + n_sub], f32, isOutput=False)
    out_sh = nc.declare_dram_parameter("out", [SEGS_PER_CORE, D], f32, isOutput=True)

    from contextlib import ExitStack
    with TileContext(nc) as tc:
        with ExitStack() as stk:
            ec = stk.enter_context
            cpool = ec(tc.tile_pool(name="consts", bufs=1))
            xapool = ec(tc.tile_pool(name="xa", bufs=18))
            xtpool = ec(tc.tile_pool(name="xt", bufs=20))
            thpool = ec(tc.tile_pool(name="th", bufs=10))
            e4pool = ec(tc.tile_pool(name="e4", bufs=6))
            oepool = ec(tc.tile_pool(name="oe", bufs=24))
            ufpool = ec(tc.tile_pool(name="uflush", bufs=2))
            sutpool = ec(tc.tile_pool(name="sut", bufs=2))
            rdpool = ec(tc.tile_pool(name="rd", bufs=2))
            osbpool = ec(tc.tile_pool(name="osb", bufs=2))
            # ---- constants into SBUF (3 DMAs)
            c16 = cpool.tile([128, C16_END], f16, tag="c16")
            nc.sync.dma_start(out=c16[:, :], in_=data[n_super, :, 0:C16_END])
            c32 = cpool.tile([128, C32_REL + n_sub], f32, tag="c32")
            nc.sync.dma_start(out=c32[:, :], in_=c32in[:, :])
            w18 = cpool.tile([128, 2, 512], f8, tag="w18")
            nc.sync.dma_start(out=w18[:, :, :], in_=data8[n_super])

            w2c = c16[:, C16_W2:C16_W2 + 2]
            iota = c16[:, C16_IOTA:C16_IOTA + 128]
            wpsb = c32[:, C32_WP:C32_WP + 512]
            bpb = c32[:, C32_BP:C32_BP + 256]
            b1c = c32[:, C32_B1:C32_B1 + 2]
            ebias = c32[:, C32_EB:C32_EB + 1]
            idf = c32[:, C32_IDF:C32_IDF + 128]

            phpool = ec(tc.tile_pool(name="ph", bufs=4, space="PSUM"))
            pupool = ec(tc.tile_pool(name="pu", bufs=2, space="PSUM"))
            eppool = ec(tc.tile_pool(name="ep", bufs=1, space="PSUM"))
            ps4pool = ec(tc.tile_pool(name="ps4", bufs=1, space="PSUM"))

            SG_EFF = max(1, int(SG * WORK_FRAC))
            for g in range(GROUPS):
                pu = pupool.tile([128, 257], f32, tag="pu")
                ps4b = ps4pool.tile([128, 16], f32, tag="ps4")
                rel_sb = c32[:, C32_REL + g * SG * SPS : C32_REL + (g + 1) * SG * SPS]
                xnats = [None] * 4
                last_flushed = -1
                for it in range(SG_EFF):
                    sidx = g * SG + it           # super-tile index
                    xt8 = xtpool.tile([128, 2, 512], f8, tag="xt8")
                    nc.sync.dma_start(out=xt8[:, :, :], in_=data8[sidx])
                    xa = xapool.tile([128, WCH], f16, tag="xa")
                    nc.sync.dma_start(out=xa[:, :], in_=data[sidx, :, :])
                    xnats[it % 4] = xa

                    # hT = W1^T x^T: one fp8 DoubleRow matmul per dout block
                    # (K=256 contracted in a single pass)
                    ph0 = phpool.tile([128, 512], f32, tag="ph")
                    ph1 = phpool.tile([128, 512], f32, tag="ph")
                    for dblk, ph in ((0, ph0), (1, ph1)):
                        nc.tensor.matmul(
                            ph[:, :],
                            lhsT=w18[:, :, dblk * 128 : (dblk + 1) * 128],
                            rhs=xt8[:, :, :],
                            start=True,
                            stop=True,
                            perf_mode=DR,
                        )
                    # tanh(h/8 + b1)  (ACT; undoes the x8 fp8 W1 scaling)
                    th0 = thpool.tile([128, 512], f16, tag="th0")
                    th1 = thpool.tile([128, 512], f16, tag="th1")
                    nc.scalar.activation(
                        th0[:, :], ph0[:, :], Act.Tanh,
                        bias=b1c[:, 0:1], scale=1.0 / W1SCALE,
                    )
                    nc.scalar.activation(
                        th1[:, :], ph1[:, :], Act.Tanh,
                        bias=b1c[:, 1:2], scale=1.0 / W1SCALE,
                    )

                    # s^T columns: ps4[node, j] = sum_dout th[dout, node] w2[dout]
                    # (own PSUM bank - a start=True matmul marks its whole 2KB
                    # zero-region pending, so it must not share a bank with the
                    # long-lived pu accumulator; 4 regions, one exp per 4 supers)
                    ps4 = ps4b[:, 4 * (it % 4) : 4 * (it % 4) + 4]
                    for j in range(SPS):
                        nc.tensor.matmul(
                            ps4[:, j : j + 1],
                            lhsT=th0[:, j * 128 : (j + 1) * 128],
                            rhs=w2c[:, 0:1],
                            start=True,
                            stop=False,
                            skip_group_check=True,
                        )
                        nc.tensor.matmul(
                            ps4[:, j : j + 1],
                            lhsT=th1[:, j * 128 : (j + 1) * 128],
                            rhs=w2c[:, 1:2],
                            start=False,
                            stop=True,
                            skip_group_check=True,
                        )

                    # e = exp(s + b2 - C), batched over up to 4 supers;
                    # flush per-super near the group end to keep the epilogue
                    # off the batched critical path
                    if it % 4 == 3 or it >= SG_EFF - 4:
                        b0 = last_flushed + 1
                        e4b = e4pool.tile([128, 4 * SPS], f32, tag="e4")
                        nc.scalar.activation(
                            e4b[:, 4 * (b0 % 4) : 4 * (it % 4) + 4],
                            ps4b[:, 4 * (b0 % 4) : 4 * (it % 4) + 4],
                            Act.Exp,
                            bias=ebias[:, 0:1],
                        )
                        pend = list(range(b0, it + 1))
                        last_flushed = it
                    else:
                        pend = None

                    # per subtile: Oe = (iota==rel) * e ; U += Oe^T @ [x|1]
                    if pend is None:
                        continue
                    for pit in pend:
                        xnat = xnats[pit % 4]
                        relbase = pit * SPS
                        for j in range(SPS):
                            oe = oepool.tile([128, 128], f16, tag="oe")
                            nc.vector.tensor_scalar(
                                out=oe[:, :],
                                in0=iota[:, :],
                                scalar1=rel_sb[:, relbase + j : relbase + j + 1],
                                scalar2=e4b[:, 4 * (pit % 4) + j : 4 * (pit % 4) + j + 1],
                                op0=Alu.is_equal,
                                op1=Alu.mult,
                            )
                            # ones column interleaved: one matmul covers
                            # U[:,0:256] and the count column U[:,256]
                            nc.tensor.matmul(
                                pu[:, 0:257],
                                lhsT=oe[:, :],
                                rhs=xnat[:, j * 257 : (j + 1) * 257],
                                start=(pit == 0 and j == 0),
                                stop=(pit == SG_EFF - 1 and j == SPS - 1),
                                skip_group_check=True,
                            )
                # flush group chunk to SBUF
                uf = ufpool.tile([128, 257], f32, tag="uf")
                nc.vector.tensor_copy(out=uf[:, :], in_=pu[:, 0:257])
                # epilogue for this group: out = (U @ Wp) / denom + bp
                ep = eppool.tile([128, 512], f32, tag="ep")
                put = ep[:, 0:256]
                nc.tensor.transpose(put[:, 0:128], uf[:, 0:128], idf)
                nc.tensor.transpose(put[:, 128:256], uf[:, 128:256], idf)
                sut = sutpool.tile([128, 256], f32, tag="sut")
                nc.vector.tensor_copy(out=sut[:, :], in_=put[:, :])
                po = ep[:, 256:512]
                nc.tensor.matmul(po[:, :], lhsT=sut[:, 0:128], rhs=wpsb[:, 0:256], start=True, stop=False)
                nc.tensor.matmul(po[:, :], lhsT=sut[:, 128:256], rhs=wpsb[:, 256:512], start=False, stop=True)
                rd = rdpool.tile([128, 1], f32, tag="rd")
                nc.vector.reciprocal(out=rd[:, :], in_=uf[:, 256:257])
                osb = osbpool.tile([128, 256], f32, tag="osb")
                nc.vector.scalar_tensor_tensor(
                    out=osb[:, :],
                    in0=po[:, :],
                    scalar=rd[:, 0:1],
                    in1=bpb[:, :],
                    op0=Alu.mult,
                    op1=Alu.add,
                )
                nc.sync.dma_start(
                    out=out_sh[g * 128 : (g + 1) * 128, :], in_=osb[:, :]
                )
    return nc


def _prepare(x, batch, W1, b1, w2, b2, Wp, bp):
    _patch_drain()
    _fast_dispatch()
    import ml_dtypes
    f8np = ml_dtypes.float8_e4m3

    x = np.asarray(x, dtype=np.float32)
    batch_np = np.asarray(batch).astype(np.int64)
    W1 = np.asarray(W1, dtype=np.float32)
    b1 = np.asarray(b1, dtype=np.float32)
    w2 = np.asarray(w2, dtype=np.float32)
    b2 = float(np.asarray(b2))
    Wp = np.asarray(Wp, dtype=np.float32)
    bp = np.asarray(bp, dtype=np.float32)

    n, d = x.shape
    assert (n, d) == (N, D)

    # piece p (p = 0..31): nodes whose segment is in [128p, 128(p+1))
    bounds = np.searchsorted(batch_np, np.arange(0, B + 1, CHUNK))  # [33]
    piece_nodes = np.diff(bounds)
    SG = int(np.ceil(piece_nodes.max() / (SPS * SUB)))
    n_super = GROUPS * SG
    n_sub = n_super * SPS

    nc = _build_nc(SG)

    f16 = np.float16
    # ---- consts16 chunk (shared by all cores)
    c16 = np.zeros((SUB, WCH), dtype=f16)
    c16[:, C16_W2:C16_W2 + 2] = np.stack([w2[0:128], w2[128:256]], axis=1).astype(f16)
    c16[:, C16_IOTA:C16_IOTA + 128] = np.tile(
        np.arange(128, dtype=f16)[None, :], (128, 1)
    )
    c16[:, C16_ONE:C16_ONE + 1] = 1.0

    # ---- fp8 W1 chunk: [p, t, c] = 8*W1[t*128+p, c]
    w18 = np.zeros((SUB, 2, 512), dtype=f8np)
    w18[:, :, 0:256] = (
        (W1SCALE * W1).reshape(2, 128, 256).transpose(1, 0, 2).astype(f8np)
    )

    # ---- consts32 (wpsb | bpb | b1c | ebias | idf | relT), rel is per-core
    c32_base = np.zeros((128, C32_REL + n_sub), dtype=np.float32)
    c32_base[:, C32_WP:C32_WP + 256] = Wp[0:128, :]
    c32_base[:, C32_WP + 256:C32_WP + 512] = Wp[128:256, :]
    c32_base[:, C32_BP:C32_BP + 256] = np.tile(bp[None, :], (128, 1))
    c32_base[:, C32_B1:C32_B1 + 2] = np.stack([b1[0:128], b1[128:256]], axis=1)
    c32_base[:, C32_EB] = b2 - C_OFF
    c32_base[:, C32_IDF:C32_IDF + 128] = np.eye(128, dtype=np.float32)

    x16 = x.astype(f16)

    in_maps = []
    for c in range(NCORES):
        xflat = np.zeros((n_sub * SUB, D), dtype=f16)
        rel_c = np.full(n_sub * SUB, -1.0, dtype=np.float32)
        for g in range(GROUPS):
            p = c * GROUPS + g
            plo, phi = int(bounds[p]), int(bounds[p + 1])
            npc = phi - plo
            off = g * SG * SPS * SUB
            xflat[off : off + npc] = x16[plo:phi]
            rel_c[off : off + npc] = (batch_np[plo:phi] - (p * CHUNK)).astype(np.float32)

        # fp8 xT chunks in DoubleRow layout: [s, p, t, n] = x[node n, t*128+p]
        data8_c = np.zeros((n_super + 1, SUB, 2, 512), dtype=f8np)
        data8_c[:n_super] = (
            xflat.reshape(n_super, SPS * SUB, 2, 128)
            .transpose(0, 3, 2, 1)
            .astype(f8np)
        )
        data8_c[n_super] = w18

        # natural-x chunks, ones col interleaved per subtile
        data_c = np.zeros((n_super + 1, SUB, WCH), dtype=f16)
        xnat_all = xflat.reshape(n_super, SPS, SUB, D).transpose(0, 2, 1, 3)
        dview = data_c[:n_super].reshape(n_super, SUB, SPS, 257)
        dview[:, :, :, 0:256] = xnat_all
        dview[:, :, :, 256] = 1.0
        data_c[n_super] = c16

        c32_c = c32_base.copy()
        c32_c[:, C32_REL:] = rel_c.reshape(n_sub, SUB).T  # [128, n_sub]

        in_maps.append({"data": data_c, "data8": data8_c, "c32": c32_c})

    return nc, in_maps


def kernel(x, batch, W1, b1, w2, b2, Wp, bp):
    from concourse.bass_utils import run_bass_kernel_spmd

    nc, in_maps = _prepare(x, batch, W1, b1, w2, b2, Wp, bp)
    import kernel as _self
    res = run_bass_kernel_spmd(nc, in_maps, core_ids=list(range(NCORES)))
    _self._last_res = res
    out = np.concatenate([res.results[c]["out"] for c in range(NCORES)], axis=0)
    return out.astype(np.float32)


# revision 5
# speedup vs baseline: 1.0134x; 1.0134x over previous
import numpy as np

# nn_AttentionPooling: pooled = segsum(softmax_seg(MLP(x)) * x) @ Wp + bp
# N=1M nodes, D=256, B=4096 segments, batch sorted. 8 NeuronCores.
#
# Strategy: shard nodes at segment boundaries so core c owns segments
# [512c, 512(c+1)) exactly -> the segment reduction is fully core-local and
# no collective is needed. Within a core, nodes are further split at every
# 128-segment boundary into 4 "groups"; each group accumulates a PSUM chunk
# U[128 segs, 256+1] via one-hot weighted matmuls (one-hot built on-device
# from host-precomputed relative segment ids). exp(s) is computed with a
# fixed offset C instead of the per-segment max (mathematically identical
# softmax; s is bounded by ||w2||_1 so no overflow).
#
# v4: the per-exec dispatch overhead scales with the number of input
# tensors (~36us each) and with the effectful (slow-path) jax dispatch.
# All inputs are packed into two tensors (one f16 "data" blob of uniform
# [128,1028] chunks + one f32 consts/rel blob), and bass_fast_dispatch
# is enabled so jit execs take the C++ fast path. The natural layout
# carries an interleaved ones column per subtile so the denominator
# rides the pooling matmul (1 matmul per subtile).
# v6: both layouts ship for ALL supers (no on-device transposes) - the
# ~450ns/super of PE transpose+LDW work costs more than the extra
# ~790ns/super of DMA given PE is the bottleneck engine. fp8/DoubleRow
# was tried and measured 1.96e-2 max rel err (outlier segments see the
# full fp8 quantization noise, no sqrt(n) averaging) - too close to the
# 2e-2 gate, so everything stays fp16.

N = 1_000_000
D = 256
B = 4096
NCORES = 8
SEGS_PER_CORE = B // NCORES          # 512
CHUNK = 128                          # segments per PSUM chunk
GROUPS = SEGS_PER_CORE // CHUNK      # 4
SUB = 128                            # nodes per subtile (partition dim)
SPS = 4                              # subtiles per super-tile
C_OFF = 4.0                          # exp(s - C_OFF) for range safety
WCH = SPS * 257                      # data chunk width (1028)

_patched = False
WORK_FRAC = 1.0  # debug knob: fraction of super-tiles emitted (timing experiments)


def _patch_drain():
    """walrus core_v3 allows 1 sync-wait per CTRL drain; split Tile's tail
    drain waits across a chain of drains."""
    global _patched
    if _patched:
        return
    import concourse.tile as tile_mod

    def _split_drain_and_barrier(self, tick_clock, wait_clock):
        drain_inst = self.nc.sync.drain()
        wait_clock.add_sem_waits(
            drain_inst.ins, tile_mod.ScopedClock({None: tick_clock.global_clock})
        )
        si = drain_inst.ins.sync_info
        if si is not None and si.on_wait is not None and len(si.on_wait) > 1:
            waits = list(si.on_wait)
            SI = type(si)
            si.on_wait = waits[:1]
            for w in waits[1:]:
                extra = self.nc.sync.drain()
                extra.ins.sync_info = SI(on_wait=[w], on_update=[])
        self.nc.all_engine_barrier()
        assert self.sems is not None
        popped = self.nc._tile_sem_poison_stack.pop()
        assert popped is self._sem_poison
        self.nc.clear_and_free_semaphores(list(self.sems.allocated().values()))
        self.nc.all_engine_barrier()

    tile_mod.TileContext._drain_and_barrier = _split_drain_and_barrier

    # Split >1-wait instructions: walrus codegen has tiny per-instruction
    # sync-wait caps. Insert same-engine NOPs carrying the excess waits.
    import concourse.mybir as mybir
    _orig_lower = tile_mod.TileContext._lower_ordered_insts

    def _lower_with_wait_split(self, ordered):
        for bbname in list(ordered.keys()):
            insts = ordered[bbname]
            newl = []
            for inst in insts:
                si = getattr(inst, "sync_info", None)
                eng = getattr(inst, "engine", None)
                ow = list(si.on_wait) if (si is not None and si.on_wait) else []
                if (
                    len(ow) > 1
                    and eng is not None
                    and eng in self.nc.engines
                    and not isinstance(inst, tile_mod.TileBranchInst)
                ):
                    SI = type(si)
                    si.on_wait = ow[-1:]
                    for w in ow[:-1]:
                        nop = self.nc.engines[eng].nop(nofuse=True, hint="wsplit")
                        nop.ins.sync_info = SI(on_wait=[w], on_update=[])
                        newl.append(nop.ins)
                newl.append(inst)
            ordered[bbname] = newl
        return _orig_lower(self, ordered)

    tile_mod.TileContext._lower_ordered_insts = _lower_with_wait_split
    _patched = True


def _fast_dispatch():
    """Enable concourse's C++ fast-path jit dispatch (drops the bass_exec
    effect token). Roughly halves the per-exec python dispatch overhead on
    the axon client; computation semantics are unchanged."""
    try:
        import jax
        import concourse.bass2jax  # noqa: F401  (registers the config)
        jax.config.update("bass_fast_dispatch", True)
    except Exception:
        pass


# consts16 column offsets (inside the last f16 data chunk)
C16_W1, C16_W2, C16_IOTA, C16_ONE = 0, 512, 514, 642
C16_END = 643
# consts32 column offsets
C32_WP, C32_BP, C32_B1, C32_EB, C32_IDF, C32_REL = 0, 512, 768, 770, 771, 899


def _build_nc(n_super_per_group):
    import concourse.bass as bass
    import concourse.mybir as mybir
    from concourse.tile import TileContext

    dt = mybir.dt
    f32 = dt.float32
    f16 = dt.float16
    Alu = mybir.AluOpType
    Act = mybir.ActivationFunctionType

    SG = n_super_per_group
    n_super = GROUPS * SG
    n_sub = n_super * SPS

    nc = bass.Bass(target_bir_lowering=False, use_seq_codegen=True)

    # f16 blob: xT chunks | natural-x chunks (ones col interleaved) | consts
    data = nc.declare_dram_parameter(
        "data", [2 * n_super + 1, SUB, WCH], f16, isOutput=False
    )
    c32in = nc.declare_dram_parameter("c32", [128, C32_REL + n_sub], f32, isOutput=False)
    out_sh = nc.declare_dram_parameter("out", [SEGS_PER_CORE, D], f32, isOutput=True)

    from contextlib import ExitStack
    with TileContext(nc) as tc:
        with ExitStack() as stk:
            ec = stk.enter_context
            cpool = ec(tc.tile_pool(name="consts", bufs=1))
            xapool = ec(tc.tile_pool(name="xa", bufs=18))
            xtpool = ec(tc.tile_pool(name="xt", bufs=20))
            thpool = ec(tc.tile_pool(name="th", bufs=10))
            e4pool = ec(tc.tile_pool(name="e4", bufs=6))
            oepool = ec(tc.tile_pool(name="oe", bufs=24))
            ufpool = ec(tc.tile_pool(name="uflush", bufs=2))
            sutpool = ec(tc.tile_pool(name="sut", bufs=2))
            rdpool = ec(tc.tile_pool(name="rd", bufs=2))
            osbpool = ec(tc.tile_pool(name="osb", bufs=2))
            # ---- constants into SBUF (2 DMAs)
            c16 = cpool.tile([128, C16_END], f16, tag="c16")
            nc.sync.dma_start(out=c16[:, :], in_=data[2 * n_super, :, 0:C16_END])
            c32 = cpool.tile([128, C32_REL + n_sub], f32, tag="c32")
            nc.sync.dma_start(out=c32[:, :], in_=c32in[:, :])

            w1sb = c16[:, C16_W1:C16_W1 + 512]
            w2c = c16[:, C16_W2:C16_W2 + 2]
            iota = c16[:, C16_IOTA:C16_IOTA + 128]
            wpsb = c32[:, C32_WP:C32_WP + 512]
            bpb = c32[:, C32_BP:C32_BP + 256]
            b1c = c32[:, C32_B1:C32_B1 + 2]
            ebias = c32[:, C32_EB:C32_EB + 1]
            idf = c32[:, C32_IDF:C32_IDF + 128]

            phpool = ec(tc.tile_pool(name="ph", bufs=4, space="PSUM"))
            pupool = ec(tc.tile_pool(name="pu", bufs=2, space="PSUM"))
            eppool = ec(tc.tile_pool(name="ep", bufs=1, space="PSUM"))
            ps4pool = ec(tc.tile_pool(name="ps4", bufs=1, space="PSUM"))

            SG_EFF = max(1, int(SG * WORK_FRAC))
            for g in range(GROUPS):
                pu = pupool.tile([128, 257], f32, tag="pu")
                ps4b = ps4pool.tile([128, 16], f32, tag="ps4")
                rel_sb = c32[:, C32_REL + g * SG * SPS : C32_REL + (g + 1) * SG * SPS]
                xnats = [None] * 4
                last_flushed = -1
                for it in range(SG_EFF):
                    sidx = g * SG + it           # super-tile index
                    xt = xtpool.tile([128, 1024], f16, tag="xt")
                    nc.sync.dma_start(out=xt[:, :], in_=data[sidx, :, 0:1024])
                    xa = xapool.tile([128, WCH], f16, tag="xa")
                    nc.sync.dma_start(out=xa[:, :], in_=data[n_super + sidx, :, :])
                    xnats[it % 4] = xa

                    # hT = W1^T x^T  (2 dout blocks x 2 k blocks)
                    ph0 = phpool.tile([128, 512], f32, tag="ph")
                    ph1 = phpool.tile([128, 512], f32, tag="ph")
                    for dblk, ph in ((0, ph0), (1, ph1)):
                        for k in range(2):
                            nc.tensor.matmul(
                                ph[:, :],
                                lhsT=w1sb[:, (2 * k + dblk) * 128 : (2 * k + dblk + 1) * 128],
                                rhs=xt[:, k * 512 : (k + 1) * 512],
                                start=(k == 0),
                                stop=(k == 1),
                            )
                    # tanh(h + b1)  (ACT, per-partition bias)
                    th0 = thpool.tile([128, 512], f16, tag="th0")
                    th1 = thpool.tile([128, 512], f16, tag="th1")
                    nc.scalar.activation(th0[:, :], ph0[:, :], Act.Tanh, bias=b1c[:, 0:1])
                    nc.scalar.activation(th1[:, :], ph1[:, :], Act.Tanh, bias=b1c[:, 1:2])

                    # s^T columns: ps4[node, j] = sum_dout th[dout, node] w2[dout]
                    # (own PSUM bank - a start=True matmul marks its whole 2KB
                    # zero-region pending, so it must not share a bank with the
                    # long-lived pu accumulator; 4 regions, one exp per 4 supers)
                    ps4 = ps4b[:, 4 * (it % 4) : 4 * (it % 4) + 4]
                    for j in range(SPS):
                        nc.tensor.matmul(
                            ps4[:, j : j + 1],
                            lhsT=th0[:, j * 128 : (j + 1) * 128],
                            rhs=w2c[:, 0:1],
                            start=True,
                            stop=False,
                            skip_group_check=True,
                        )
                        nc.tensor.matmul(
                            ps4[:, j : j + 1],
                            lhsT=th1[:, j * 128 : (j + 1) * 128],
                            rhs=w2c[:, 1:2],
                            start=False,
                            stop=True,
                            skip_group_check=True,
                        )

                    # e = exp(s + b2 - C), batched over up to 4 supers;
                    # flush per-super near the group end to keep the epilogue
                    # off the batched critical path
                    if it % 4 == 3 or it >= SG_EFF - 4:
                        b0 = last_flushed + 1
                        e4b = e4pool.tile([128, 4 * SPS], f32, tag="e4")
                        nc.scalar.activation(
                            e4b[:, 4 * (b0 % 4) : 4 * (it % 4) + 4],
                            ps4b[:, 4 * (b0 % 4) : 4 * (it % 4) + 4],
                            Act.Exp,
                            bias=ebias[:, 0:1],
                        )
                        pend = list(range(b0, it + 1))
                        last_flushed = it
                    else:
                        pend = None

                    # per subtile: Oe = (iota==rel) * e ; U += Oe^T @ [x|1]
                    if pend is None:
                        continue
                    for pit in pend:
                        xnat = xnats[pit % 4]
                        relbase = pit * SPS
                        for j in range(SPS):
                            oe = oepool.tile([128, 128], f16, tag="oe")
                            nc.vector.tensor_scalar(
                                out=oe[:, :],
                                in0=iota[:, :],
                                scalar1=rel_sb[:, relbase + j : relbase + j + 1],
                                scalar2=e4b[:, 4 * (pit % 4) + j : 4 * (pit % 4) + j + 1],
                                op0=Alu.is_equal,
                                op1=Alu.mult,
                            )
                            # ones column interleaved: one matmul covers
                            # U[:,0:256] and the count column U[:,256]
                            nc.tensor.matmul(
                                pu[:, 0:257],
                                lhsT=oe[:, :],
                                rhs=xnat[:, j * 257 : (j + 1) * 257],
                                start=(pit == 0 and j == 0),
                                stop=(pit == SG_EFF - 1 and j == SPS - 1),
                                skip_group_check=True,
                            )
                # flush group chunk to SBUF
                uf = ufpool.tile([128, 257], f32, tag="uf")
                nc.vector.tensor_copy(out=uf[:, :], in_=pu[:, 0:257])
                # epilogue for this group: out = (U @ Wp) / denom + bp
                ep = eppool.tile([128, 512], f32, tag="ep")
                put = ep[:, 0:256]
                nc.tensor.transpose(put[:, 0:128], uf[:, 0:128], idf)
                nc.tensor.transpose(put[:, 128:256], uf[:, 128:256], idf)
                sut = sutpool.tile([128, 256], f32, tag="sut")
                nc.vector.tensor_copy(out=sut[:, :], in_=put[:, :])
                po = ep[:, 256:512]
                nc.tensor.matmul(po[:, :], lhsT=sut[:, 0:128], rhs=wpsb[:, 0:256], start=True, stop=False)
                nc.tensor.matmul(po[:, :], lhsT=sut[:, 128:256], rhs=wpsb[:, 256:512], start=False, stop=True)
                rd = rdpool.tile([128, 1], f32, tag="rd")
                nc.vector.reciprocal(out=rd[:, :], in_=uf[:, 256:257])
                osb = osbpool.tile([128, 256], f32, tag="osb")
                nc.vector.scalar_tensor_tensor(
                    out=osb[:, :],
                    in0=po[:, :],
                    scalar=rd[:, 0:1],
                    in1=bpb[:, :],
                    op0=Alu.mult,
                    op1=Alu.add,
                )
                nc.sync.dma_start(
                    out=out_sh[g * 128 : (g + 1) * 128, :], in_=osb[:, :]
                )
    return nc


def _prepare(x, batch, W1, b1, w2, b2, Wp, bp):
    _patch_drain()
    _fast_dispatch()

    x = np.asarray(x, dtype=np.float32)
    batch_np = np.asarray(batch).astype(np.int64)
    W1 = np.asarray(W1, dtype=np.float32)
    b1 = np.asarray(b1, dtype=np.float32)
    w2 = np.asarray(w2, dtype=np.float32)
    b2 = float(np.asarray(b2))
    Wp = np.asarray(Wp, dtype=np.float32)
    bp = np.asarray(bp, dtype=np.float32)

    n, d = x.shape
    assert (n, d) == (N, D)

    # piece p (p = 0..31): nodes whose segment is in [128p, 128(p+1))
    bounds = np.searchsorted(batch_np, np.arange(0, B + 1, CHUNK))  # [33]
    piece_nodes = np.diff(bounds)
    SG = int(np.ceil(piece_nodes.max() / (SPS * SUB)))
    n_super = GROUPS * SG
    n_sub = n_super * SPS

    nc = _build_nc(SG)

    f16 = np.float16
    # ---- consts16 chunk (shared by all cores)
    c16 = np.zeros((SUB, WCH), dtype=f16)
    for k in range(2):
        for dblk in range(2):
            c16[:, C16_W1 + (2 * k + dblk) * 128 : C16_W1 + (2 * k + dblk + 1) * 128] = (
                W1[k * 128 : (k + 1) * 128, dblk * 128 : (dblk + 1) * 128]
            ).astype(f16)
    c16[:, C16_W2:C16_W2 + 2] = np.stack([w2[0:128], w2[128:256]], axis=1).astype(f16)
    c16[:, C16_IOTA:C16_IOTA + 128] = np.tile(
        np.arange(128, dtype=f16)[None, :], (128, 1)
    )
    c16[:, C16_ONE:C16_ONE + 1] = 1.0

    # ---- consts32 (wpsb | bpb | b1c | ebias | idf | relT), rel is per-core
    c32_base = np.zeros((128, C32_REL + n_sub), dtype=np.float32)
    c32_base[:, C32_WP:C32_WP + 256] = Wp[0:128, :]
    c32_base[:, C32_WP + 256:C32_WP + 512] = Wp[128:256, :]
    c32_base[:, C32_BP:C32_BP + 256] = np.tile(bp[None, :], (128, 1))
    c32_base[:, C32_B1:C32_B1 + 2] = np.stack([b1[0:128], b1[128:256]], axis=1)
    c32_base[:, C32_EB] = b2 - C_OFF
    c32_base[:, C32_IDF:C32_IDF + 128] = np.eye(128, dtype=np.float32)

    x16 = x.astype(f16)

    in_maps = []
    for c in range(NCORES):
        xflat = np.zeros((n_sub * SUB, D), dtype=f16)
        rel_c = np.full(n_sub * SUB, -1.0, dtype=np.float32)
        for g in range(GROUPS):
            p = c * GROUPS + g
            plo, phi = int(bounds[p]), int(bounds[p + 1])
            npc = phi - plo
            off = g * SG * SPS * SUB
            xflat[off : off + npc] = x16[plo:phi]
            rel_c[off : off + npc] = (batch_np[plo:phi] - (p * CHUNK)).astype(np.float32)

        data_c = np.zeros((2 * n_super + 1, SUB, WCH), dtype=f16)
        # xT chunks: row p, col k*512+n = x[node n, din 128k+p]
        data_c[:n_super, :, 0:1024] = (
            xflat.reshape(n_super, SPS * SUB, 2, 128)
            .transpose(0, 3, 2, 1)
            .reshape(n_super, 128, 2 * SPS * SUB)
        )
        # natural-x chunks, ones col interleaved per subtile
        xnat_all = xflat.reshape(n_super, SPS, SUB, D).transpose(0, 2, 1, 3)
        dview = data_c[n_super:2 * n_super].reshape(n_super, SUB, SPS, 257)
        dview[:, :, :, 0:256] = xnat_all
        dview[:, :, :, 256] = 1.0
        data_c[2 * n_super] = c16

        c32_c = c32_base.copy()
        c32_c[:, C32_REL:] = rel_c.reshape(n_sub, SUB).T  # [128, n_sub]

        in_maps.append({"data": data_c, "c32": c32_c})

    return nc, in_maps


def kernel(x, batch, W1, b1, w2, b2, Wp, bp):
    from concourse.bass_utils import run_bass_kernel_spmd

    nc, in_maps = _prepare(x, batch, W1, b1, w2, b2, Wp, bp)
    import kernel as _self
    res = run_bass_kernel_spmd(nc, in_maps, core_ids=list(range(NCORES)))
    _self._last_res = res
    out = np.concatenate([res.results[c]["out"] for c in range(NCORES)], axis=0)
    return out.astype(np.float32)


# revision 8
# speedup vs baseline: 1.0460x; 1.0322x over previous
import numpy as np

# nn_AttentionPooling: pooled = segsum(softmax_seg(MLP(x)) * x) @ Wp + bp
# N=1M nodes, D=256, B=4096 segments, batch sorted. 8 NeuronCores.
#
# Strategy: shard nodes at segment boundaries so core c owns segments
# [512c, 512(c+1)) exactly -> the segment reduction is fully core-local and
# no collective is needed. Within a core, nodes are further split at every
# 128-segment boundary into 4 "groups"; each group accumulates a PSUM chunk
# U[128 segs, 256+1] via one-hot weighted matmuls (one-hot built on-device
# from host-precomputed relative segment ids). exp(s) is computed with a
# fixed offset C instead of the per-segment max (mathematically identical
# softmax; s is bounded by ||w2||_1 so no overflow).
#
# v4: the per-exec dispatch overhead scales with the number of input
# tensors (~36us each) and with the effectful (slow-path) jax dispatch.
# All inputs are packed into two tensors (one f16 "data" blob of uniform
# [128,1028] chunks + one f32 consts/rel blob), and bass_fast_dispatch
# is enabled so jit execs take the C++ fast path. The natural layout
# carries an interleaved ones column per subtile so the denominator
# rides the pooling matmul (1 matmul per subtile). fp8/DoubleRow was
# tried and measured 1.96e-2 max rel err (outlier segments see the full
# fp8 quantization noise) - too close to the 2e-2 gate; all fp16.
# v7: profile showed pooling matmuls cost 223ns vs their 107ns of array
# time - back-to-back 128-col stationary loads can't hide under the
# short 257-col streams. Pooling matmuls are now DEFERRED and emitted
# one per MLP matmul of later supers, so each oe LDWEIGHTS pulls ahead
# under a 512-col MLP stream. 1/4 of supers are PE-transposed on device
# (skipping their natural-layout DMA) to rebalance DMA against the
# recovered PE headroom.

N = 1_000_000
D = 256
B = 4096
NCORES = 8
SEGS_PER_CORE = B // NCORES          # 512
CHUNK = 128                          # segments per PSUM chunk
GROUPS = SEGS_PER_CORE // CHUNK      # 4
SUB = 128                            # nodes per subtile (partition dim)
SPS = 4                              # subtiles per super-tile
C_OFF = 4.0                          # exp(s - C_OFF) for range safety
WCH = SPS * 257                      # data chunk width (1028)
SHIP_NUM, SHIP_DEN = 3, 4            # fraction of supers shipped in natural layout

_patched = False
WORK_FRAC = 1.0  # debug knob: fraction of super-tiles emitted (timing experiments)


def _patch_drain():
    """walrus core_v3 allows 1 sync-wait per CTRL drain; split Tile's tail
    drain waits across a chain of drains."""
    global _patched
    if _patched:
        return
    import concourse.tile as tile_mod

    def _split_drain_and_barrier(self, tick_clock, wait_clock):
        drain_inst = self.nc.sync.drain()
        wait_clock.add_sem_waits(
            drain_inst.ins, tile_mod.ScopedClock({None: tick_clock.global_clock})
        )
        si = drain_inst.ins.sync_info
        if si is not None and si.on_wait is not None and len(si.on_wait) > 1:
            waits = list(si.on_wait)
            SI = type(si)
            si.on_wait = waits[:1]
            for w in waits[1:]:
                extra = self.nc.sync.drain()
                extra.ins.sync_info = SI(on_wait=[w], on_update=[])
        self.nc.all_engine_barrier()
        assert self.sems is not None
        popped = self.nc._tile_sem_poison_stack.pop()
        assert popped is self._sem_poison
        self.nc.clear_and_free_semaphores(list(self.sems.allocated().values()))
        self.nc.all_engine_barrier()

    tile_mod.TileContext._drain_and_barrier = _split_drain_and_barrier

    # Split >1-wait instructions: walrus codegen has tiny per-instruction
    # sync-wait caps. Insert same-engine NOPs carrying the excess waits.
    import concourse.mybir as mybir
    _orig_lower = tile_mod.TileContext._lower_ordered_insts

    def _lower_with_wait_split(self, ordered):
        for bbname in list(ordered.keys()):
            insts = ordered[bbname]
            newl = []
            for inst in insts:
                si = getattr(inst, "sync_info", None)
                eng = getattr(inst, "engine", None)
                ow = list(si.on_wait) if (si is not None and si.on_wait) else []
                if (
                    len(ow) > 1
                    and eng is not None
                    and eng in self.nc.engines
                    and not isinstance(inst, tile_mod.TileBranchInst)
                ):
                    SI = type(si)
                    si.on_wait = ow[-1:]
                    for w in ow[:-1]:
                        nop = self.nc.engines[eng].nop(nofuse=True, hint="wsplit")
                        nop.ins.sync_info = SI(on_wait=[w], on_update=[])
                        newl.append(nop.ins)
                newl.append(inst)
            ordered[bbname] = newl
        return _orig_lower(self, ordered)

    tile_mod.TileContext._lower_ordered_insts = _lower_with_wait_split
    _patched = True


def _fast_dispatch():
    """Enable concourse's C++ fast-path jit dispatch (drops the bass_exec
    effect token). Roughly halves the per-exec python dispatch overhead on
    the axon client; computation semantics are unchanged."""
    try:
        import jax
        import concourse.bass2jax  # noqa: F401  (registers the config)
        jax.config.update("bass_fast_dispatch", True)
    except Exception:
        pass


def _tr_set(SG, n_ship):
    """Positions of device-transposed supers: spread every 4th slot so the
    PE-heavy transpose work interleaves with DMA-heavy shipped supers."""
    n_tr = SG - n_ship
    slots = [i for i in range(SG) if i % 4 == 1]
    if len(slots) < n_tr:
        slots += [i for i in range(SG) if i % 4 == 3]
    return set(slots[:n_tr])


# consts16 column offsets (inside the last f16 data chunk)
C16_W1, C16_W2, C16_IOTA, C16_IDH, C16_ONE = 0, 512, 514, 642, 770
C16_END = 771
# consts32 column offsets
C32_WP, C32_BP, C32_B1, C32_EB, C32_IDF, C32_REL = 0, 512, 768, 770, 771, 899


def _build_nc(n_super_per_group):
    import concourse.bass as bass
    import concourse.mybir as mybir
    from concourse.tile import TileContext
    from collections import deque

    dt = mybir.dt
    f32 = dt.float32
    f16 = dt.float16
    Alu = mybir.AluOpType
    Act = mybir.ActivationFunctionType

    SG = n_super_per_group
    n_super = GROUPS * SG
    n_sub = n_super * SPS

    nc = bass.Bass(target_bir_lowering=False, use_seq_codegen=True)

    n_ship = (SG * SHIP_NUM + SHIP_DEN - 1) // SHIP_DEN
    n_xa = GROUPS * n_ship
    CH_C16 = n_super + n_xa

    # f16 blob: xT chunks | natural-x chunks (ones col interleaved) | consts
    data = nc.declare_dram_parameter("data", [CH_C16 + 1, SUB, WCH], f16, isOutput=False)
    c32in = nc.declare_dram_parameter("c32", [128, C32_REL + n_sub], f32, isOutput=False)
    out_sh = nc.declare_dram_parameter("out", [SEGS_PER_CORE, D], f32, isOutput=True)

    from contextlib import ExitStack
    with TileContext(nc) as tc:
        with ExitStack() as stk:
            ec = stk.enter_context
            cpool = ec(tc.tile_pool(name="consts", bufs=1))
            xapool = ec(tc.tile_pool(name="xa", bufs=18))
            xtpool = ec(tc.tile_pool(name="xt", bufs=20))
            sxtpool = ec(tc.tile_pool(name="sxt", bufs=10))
            thpool = ec(tc.tile_pool(name="th", bufs=10))
            e4pool = ec(tc.tile_pool(name="e4", bufs=6))
            oepool = ec(tc.tile_pool(name="oe", bufs=40))
            ufpool = ec(tc.tile_pool(name="uflush", bufs=2))
            sutpool = ec(tc.tile_pool(name="sut", bufs=2))
            rdpool = ec(tc.tile_pool(name="rd", bufs=2))
            osbpool = ec(tc.tile_pool(name="osb", bufs=2))
            # ---- constants into SBUF (2 DMAs)
            c16 = cpool.tile([128, C16_END], f16, tag="c16")
            nc.sync.dma_start(out=c16[:, :], in_=data[CH_C16, :, 0:C16_END])
            c32 = cpool.tile([128, C32_REL + n_sub], f32, tag="c32")
            nc.sync.dma_start(out=c32[:, :], in_=c32in[:, :])

            w1sb = c16[:, C16_W1:C16_W1 + 512]
            w2c = c16[:, C16_W2:C16_W2 + 2]
            iota = c16[:, C16_IOTA:C16_IOTA + 128]
            idh = c16[:, C16_IDH:C16_IDH + 128]
            onescol = c16[:, C16_ONE:C16_ONE + 1]
            wpsb = c32[:, C32_WP:C32_WP + 512]
            bpb = c32[:, C32_BP:C32_BP + 256]
            b1c = c32[:, C32_B1:C32_B1 + 2]
            ebias = c32[:, C32_EB:C32_EB + 1]
            idf = c32[:, C32_IDF:C32_IDF + 128]

            phpool = ec(tc.tile_pool(name="ph", bufs=3, space="PSUM"))
            pupool = ec(tc.tile_pool(name="pu", bufs=2, space="PSUM"))
            eppool = ec(tc.tile_pool(name="ep", bufs=1, space="PSUM"))
            pxtpool = ec(tc.tile_pool(name="pxt", bufs=1, space="PSUM"))
            ps4pool = ec(tc.tile_pool(name="ps4", bufs=1, space="PSUM"))

            SG_EFF = max(1, int(SG * WORK_FRAC))
            trs = _tr_set(SG, n_ship) if SG_EFF == SG else set()

            # deferred pooling matmuls: emitted one per MLP matmul of later
            # supers so each oe LDWEIGHTS hides under a 512-col MLP stream
            pq = deque()          # (group, emit_fn)
            pend_cnt = [0] * GROUPS
            group_done = [False] * GROUPS

            def emit_epilogue(g, pu):
                # out = (U @ Wp) / denom + bp  for this group's 128 segments
                uf = ufpool.tile([128, 257], f32, tag="uf")
                nc.vector.tensor_copy(out=uf[:, :], in_=pu[:, 0:257])
                ep = eppool.tile([128, 512], f32, tag="ep")
                put = ep[:, 0:256]
                nc.tensor.transpose(put[:, 0:128], uf[:, 0:128], idf)
                nc.tensor.transpose(put[:, 128:256], uf[:, 128:256], idf)
                sut = sutpool.tile([128, 256], f32, tag="sut")
                nc.vector.tensor_copy(out=sut[:, :], in_=put[:, :])
                po = ep[:, 256:512]
                nc.tensor.matmul(po[:, :], lhsT=sut[:, 0:128], rhs=wpsb[:, 0:256], start=True, stop=False)
                nc.tensor.matmul(po[:, :], lhsT=sut[:, 128:256], rhs=wpsb[:, 256:512], start=False, stop=True)
                rd = rdpool.tile([128, 1], f32, tag="rd")
                nc.vector.reciprocal(out=rd[:, :], in_=uf[:, 256:257])
                osb = osbpool.tile([128, 256], f32, tag="osb")
                nc.vector.scalar_tensor_tensor(
                    out=osb[:, :],
                    in0=po[:, :],
                    scalar=rd[:, 0:1],
                    in1=bpb[:, :],
                    op0=Alu.mult,
                    op1=Alu.add,
                )
                nc.sync.dma_start(
                    out=out_sh[g * 128 : (g + 1) * 128, :], in_=osb[:, :]
                )

            def pool_slot():
                if not pq:
                    return
                g2, fn = pq.popleft()
                fn()
                pend_cnt[g2] -= 1
                if pend_cnt[g2] == 0 and group_done[g2]:
                    emit_epilogue(g2, pu_of[g2])

            pu_of = {}
            for g in range(GROUPS):
                pu = pupool.tile([128, 257], f32, tag="pu")
                pu_of[g] = pu
                ps4b = ps4pool.tile([128, 16], f32, tag="ps4")
                rel_sb = c32[:, C32_REL + g * SG * SPS : C32_REL + (g + 1) * SG * SPS]
                xnats = [None] * 4
                xkinds = [None] * 4
                last_flushed = -1
                ship_i = 0
                for it in range(SG_EFF):
                    sidx = g * SG + it           # super-tile index
                    xt = xtpool.tile([128, 1024], f16, tag="xt")
                    nc.sync.dma_start(out=xt[:, :], in_=data[sidx, :, 0:1024])
                    if it not in trs:
                        xa = xapool.tile([128, WCH], f16, tag="xa")
                        nc.sync.dma_start(
                            out=xa[:, :], in_=data[n_super + g * n_ship + ship_i, :, :]
                        )
                        ship_i += 1
                        xnats[it % 4] = xa
                        xkinds[it % 4] = "xa"
                    else:
                        pxt = pxtpool.tile([128, 1024], f16, tag="pxt")
                        for j in range(SPS):
                            for kb in range(2):
                                nc.tensor.transpose(
                                    pxt[:, j * 256 + kb * 128 : j * 256 + (kb + 1) * 128],
                                    xt[:, kb * 512 + j * 128 : kb * 512 + (j + 1) * 128],
                                    idh,
                                )
                        sxt = sxtpool.tile([128, 1024], f16, tag="sxt")
                        nc.vector.tensor_copy(out=sxt[:, :], in_=pxt[:, :])
                        xnats[it % 4] = sxt
                        xkinds[it % 4] = "sxt"

                    # hT = W1^T x^T (2 dout blocks x 2 k blocks); one deferred
                    # pooling matmul rides after each MLP matmul
                    ph0 = phpool.tile([128, 512], f32, tag="ph")
                    ph1 = phpool.tile([128, 512], f32, tag="ph")
                    for dblk, ph in ((0, ph0), (1, ph1)):
                        for k in range(2):
                            nc.tensor.matmul(
                                ph[:, :],
                                lhsT=w1sb[:, (2 * k + dblk) * 128 : (2 * k + dblk + 1) * 128],
                                rhs=xt[:, k * 512 : (k + 1) * 512],
                                start=(k == 0),
                                stop=(k == 1),
                            )
                            pool_slot()
                    # tanh(h + b1)  (ACT, per-partition bias)
                    th0 = thpool.tile([128, 512], f16, tag="th0")
                    th1 = thpool.tile([128, 512], f16, tag="th1")
                    nc.scalar.activation(th0[:, :], ph0[:, :], Act.Tanh, bias=b1c[:, 0:1])
                    nc.scalar.activation(th1[:, :], ph1[:, :], Act.Tanh, bias=b1c[:, 1:2])

                    # s^T columns: ps4[node, j] = sum_dout th[dout, node] w2[dout]
                    # (own PSUM bank - a start=True matmul marks its whole 2KB
                    # zero-region pending, so it must not share a bank with the
                    # long-lived pu accumulator; 4 regions, one exp per 4 supers)
                    ps4 = ps4b[:, 4 * (it % 4) : 4 * (it % 4) + 4]
                    for j in range(SPS):
                        nc.tensor.matmul(
                            ps4[:, j : j + 1],
                            lhsT=th0[:, j * 128 : (j + 1) * 128],
                            rhs=w2c[:, 0:1],
                            start=True,
                            stop=False,
                            skip_group_check=True,
                        )
                        nc.tensor.matmul(
                            ps4[:, j : j + 1],
                            lhsT=th1[:, j * 128 : (j + 1) * 128],
                            rhs=w2c[:, 1:2],
                            start=False,
                            stop=True,
                            skip_group_check=True,
                        )

                    # e = exp(s + b2 - C), batched over up to 4 supers
                    if it % 4 == 3 or it == SG_EFF - 1:
                        b0 = last_flushed + 1
                        e4b = e4pool.tile([128, 4 * SPS], f32, tag="e4")
                        nc.scalar.activation(
                            e4b[:, 4 * (b0 % 4) : 4 * (it % 4) + 4],
                            ps4b[:, 4 * (b0 % 4) : 4 * (it % 4) + 4],
                            Act.Exp,
                            bias=ebias[:, 0:1],
                        )
                        pend = list(range(b0, it + 1))
                        last_flushed = it
                    else:
                        pend = None

                    # per subtile: Oe = (iota==rel) * e (DVE, emitted now);
                    # U += Oe^T @ [x|1] (PE, deferred into pool_slot()s)
                    if pend is None:
                        continue
                    for pit in pend:
                        xnat = xnats[pit % 4]
                        kind = xkinds[pit % 4]
                        relbase = pit * SPS
                        for j in range(SPS):
                            oe = oepool.tile([128, 128], f16, tag="oe")
                            nc.vector.tensor_scalar(
                                out=oe[:, :],
                                in0=iota[:, :],
                                scalar1=rel_sb[:, relbase + j : relbase + j + 1],
                                scalar2=e4b[:, 4 * (pit % 4) + j : 4 * (pit % 4) + j + 1],
                                op0=Alu.is_equal,
                                op1=Alu.mult,
                            )
                            first = pit == 0 and j == 0
                            last = pit == SG_EFF - 1 and j == SPS - 1

                            def mk(oe=oe, xnat=xnat, kind=kind, j=j,
                                   first=first, last=last, pu=pu):
                                def emit():
                                    if kind == "xa":
                                        nc.tensor.matmul(
                                            pu[:, 0:257],
                                            lhsT=oe[:, :],
                                            rhs=xnat[:, j * 257 : (j + 1) * 257],
                                            start=first,
                                            stop=last,
                                            skip_group_check=True,
                                        )
                                    else:
                                        nc.tensor.matmul(
                                            pu[:, 0:256],
                                            lhsT=oe[:, :],
                                            rhs=xnat[:, j * 256 : (j + 1) * 256],
                                            start=first,
                                            stop=last,
                                            skip_group_check=True,
                                        )
                                        nc.tensor.matmul(
                                            pu[:, 256:257],
                                            lhsT=oe[:, :],
                                            rhs=onescol[:, 0:1],
                                            start=False,
                                            stop=last,
                                            skip_group_check=True,
                                        )
                                return emit

                            pq.append((g, mk()))
                            pend_cnt[g] += 1
                group_done[g] = True
                if pend_cnt[g] == 0:
                    emit_epilogue(g, pu_of[g])
            # drain remaining deferred pooling matmuls
            while pq:
                pool_slot()
    return nc


def _prepare(x, batch, W1, b1, w2, b2, Wp, bp):
    _patch_drain()
    _fast_dispatch()

    x = np.asarray(x, dtype=np.float32)
    batch_np = np.asarray(batch).astype(np.int64)
    W1 = np.asarray(W1, dtype=np.float32)
    b1 = np.asarray(b1, dtype=np.float32)
    w2 = np.asarray(w2, dtype=np.float32)
    b2 = float(np.asarray(b2))
    Wp = np.asarray(Wp, dtype=np.float32)
    bp = np.asarray(bp, dtype=np.float32)

    n, d = x.shape
    assert (n, d) == (N, D)

    # piece p (p = 0..31): nodes whose segment is in [128p, 128(p+1))
    bounds = np.searchsorted(batch_np, np.arange(0, B + 1, CHUNK))  # [33]
    piece_nodes = np.diff(bounds)
    SG = int(np.ceil(piece_nodes.max() / (SPS * SUB)))
    n_super = GROUPS * SG
    n_sub = n_super * SPS

    n_ship = (SG * SHIP_NUM + SHIP_DEN - 1) // SHIP_DEN
    n_xa = GROUPS * n_ship
    CH_C16 = n_super + n_xa

    nc = _build_nc(SG)

    f16 = np.float16
    # ---- consts16 chunk (shared by all cores)
    c16 = np.zeros((SUB, WCH), dtype=f16)
    for k in range(2):
        for dblk in range(2):
            c16[:, C16_W1 + (2 * k + dblk) * 128 : C16_W1 + (2 * k + dblk + 1) * 128] = (
                W1[k * 128 : (k + 1) * 128, dblk * 128 : (dblk + 1) * 128]
            ).astype(f16)
    c16[:, C16_W2:C16_W2 + 2] = np.stack([w2[0:128], w2[128:256]], axis=1).astype(f16)
    c16[:, C16_IOTA:C16_IOTA + 128] = np.tile(
        np.arange(128, dtype=f16)[None, :], (128, 1)
    )
    c16[:, C16_IDH:C16_IDH + 128] = np.eye(128, dtype=f16)
    c16[:, C16_ONE:C16_ONE + 1] = 1.0

    # ---- consts32 (wpsb | bpb | b1c | ebias | idf | relT), rel is per-core
    c32_base = np.zeros((128, C32_REL + n_sub), dtype=np.float32)
    c32_base[:, C32_WP:C32_WP + 256] = Wp[0:128, :]
    c32_base[:, C32_WP + 256:C32_WP + 512] = Wp[128:256, :]
    c32_base[:, C32_BP:C32_BP + 256] = np.tile(bp[None, :], (128, 1))
    c32_base[:, C32_B1:C32_B1 + 2] = np.stack([b1[0:128], b1[128:256]], axis=1)
    c32_base[:, C32_EB] = b2 - C_OFF
    c32_base[:, C32_IDF:C32_IDF + 128] = np.eye(128, dtype=np.float32)

    x16 = x.astype(f16)

    in_maps = []
    trs = _tr_set(SG, n_ship)
    ship_order = [it for it in range(SG) if it not in trs]
    for c in range(NCORES):
        xflat = np.zeros((n_sub * SUB, D), dtype=f16)
        rel_c = np.full(n_sub * SUB, -1.0, dtype=np.float32)
        for g in range(GROUPS):
            p = c * GROUPS + g
            plo, phi = int(bounds[p]), int(bounds[p + 1])
            npc = phi - plo
            off = g * SG * SPS * SUB
            xflat[off : off + npc] = x16[plo:phi]
            rel_c[off : off + npc] = (batch_np[plo:phi] - (p * CHUNK)).astype(np.float32)

        data_c = np.zeros((CH_C16 + 1, SUB, WCH), dtype=f16)
        # xT chunks: row p, col k*512+n = x[node n, din 128k+p]
        data_c[:n_super, :, 0:1024] = (
            xflat.reshape(n_super, SPS * SUB, 2, 128)
            .transpose(0, 3, 2, 1)
            .reshape(n_super, 128, 2 * SPS * SUB)
        )
        # natural-x chunks for shipped supers, ones col interleaved
        xnat_all = xflat.reshape(n_super, SPS, SUB, D).transpose(0, 2, 1, 3)
        ship_idx = [g * SG + it for g in range(GROUPS) for it in ship_order]
        dview = data_c[n_super:CH_C16].reshape(n_xa, SUB, SPS, 257)
        dview[:, :, :, 0:256] = xnat_all[ship_idx]
        dview[:, :, :, 256] = 1.0
        data_c[CH_C16] = c16

        c32_c = c32_base.copy()
        c32_c[:, C32_REL:] = rel_c.reshape(n_sub, SUB).T  # [128, n_sub]

        in_maps.append({"data": data_c, "c32": c32_c})

    return nc, in_maps


def kernel(x, batch, W1, b1, w2, b2, Wp, bp):
    from concourse.bass_utils import run_bass_kernel_spmd

    nc, in_maps = _prepare(x, batch, W1, b1, w2, b2, Wp, bp)
    import kernel as _self
    res = run_bass_kernel_spmd(nc, in_maps, core_ids=list(range(NCORES)))
    _self._last_res = res
    out = np.concatenate([res.results[c]["out"] for c in range(NCORES)], axis=0)
    return out.astype(np.float32)


# revision 9
# speedup vs baseline: 1.0564x; 1.0099x over previous
import numpy as np

# nn_AttentionPooling: pooled = segsum(softmax_seg(MLP(x)) * x) @ Wp + bp
# N=1M nodes, D=256, B=4096 segments, batch sorted. 8 NeuronCores.
#
# Strategy: shard nodes at segment boundaries so core c owns segments
# [512c, 512(c+1)) exactly -> the segment reduction is fully core-local and
# no collective is needed. Within a core, nodes are further split at every
# 128-segment boundary into 4 "groups"; each group accumulates a PSUM chunk
# U[128 segs, 256+1] via one-hot weighted matmuls (one-hot built on-device
# from host-precomputed relative segment ids). exp(s) is computed with a
# fixed offset C instead of the per-segment max (mathematically identical
# softmax; s is bounded by ||w2||_1 so no overflow).
#
# v4: the per-exec dispatch overhead scales with the number of input
# tensors (~36us each) and with the effectful (slow-path) jax dispatch.
# All inputs are packed into two tensors (one f16 "data" blob of uniform
# [128,1028] chunks + one f32 consts/rel blob), and bass_fast_dispatch
# is enabled so jit execs take the C++ fast path. The natural layout
# carries an interleaved ones column per subtile so the denominator
# rides the pooling matmul (1 matmul per subtile). fp8/DoubleRow was
# tried and measured 1.96e-2 max rel err (outlier segments see the full
# fp8 quantization noise) - too close to the 2e-2 gate; all fp16.
# v7: profile showed pooling matmuls cost 223ns vs their 107ns of array
# time - back-to-back 128-col stationary loads can't hide under the
# short 257-col streams. Pooling matmuls are now DEFERRED and emitted
# one per MLP matmul of later supers, so each oe LDWEIGHTS pulls ahead
# under a 512-col MLP stream. 1/4 of supers are PE-transposed on device
# (skipping their natural-layout DMA) to rebalance DMA against the
# recovered PE headroom.

N = 1_000_000
D = 256
B = 4096
NCORES = 8
SEGS_PER_CORE = B // NCORES          # 512
CHUNK = 128                          # segments per PSUM chunk
GROUPS = SEGS_PER_CORE // CHUNK      # 4
SUB = 128                            # nodes per subtile (partition dim)
SPS = 4                              # subtiles per super-tile
C_OFF = 4.0                          # exp(s - C_OFF) for range safety
WCH = SPS * 257                      # data chunk width (1028)
SHIP_NUM, SHIP_DEN = 1, 1            # fraction of supers shipped in natural layout

_patched = False
WORK_FRAC = 1.0  # debug knob: fraction of super-tiles emitted (timing experiments)


def _patch_drain():
    """walrus core_v3 allows 1 sync-wait per CTRL drain; split Tile's tail
    drain waits across a chain of drains."""
    global _patched
    if _patched:
        return
    import concourse.tile as tile_mod

    def _split_drain_and_barrier(self, tick_clock, wait_clock):
        drain_inst = self.nc.sync.drain()
        wait_clock.add_sem_waits(
            drain_inst.ins, tile_mod.ScopedClock({None: tick_clock.global_clock})
        )
        si = drain_inst.ins.sync_info
        if si is not None and si.on_wait is not None and len(si.on_wait) > 1:
            waits = list(si.on_wait)
            SI = type(si)
            si.on_wait = waits[:1]
            for w in waits[1:]:
                extra = self.nc.sync.drain()
                extra.ins.sync_info = SI(on_wait=[w], on_update=[])
        self.nc.all_engine_barrier()
        assert self.sems is not None
        popped = self.nc._tile_sem_poison_stack.pop()
        assert popped is self._sem_poison
        self.nc.clear_and_free_semaphores(list(self.sems.allocated().values()))
        self.nc.all_engine_barrier()

    tile_mod.TileContext._drain_and_barrier = _split_drain_and_barrier

    # Split >1-wait instructions: walrus codegen has tiny per-instruction
    # sync-wait caps. Insert same-engine NOPs carrying the excess waits.
    import concourse.mybir as mybir
    _orig_lower = tile_mod.TileContext._lower_ordered_insts

    def _lower_with_wait_split(self, ordered):
        for bbname in list(ordered.keys()):
            insts = ordered[bbname]
            newl = []
            for inst in insts:
                si = getattr(inst, "sync_info", None)
                eng = getattr(inst, "engine", None)
                ow = list(si.on_wait) if (si is not None and si.on_wait) else []
                if (
                    len(ow) > 1
                    and eng is not None
                    and eng in self.nc.engines
                    and not isinstance(inst, tile_mod.TileBranchInst)
                ):
                    SI = type(si)
                    si.on_wait = ow[-1:]
                    for w in ow[:-1]:
                        nop = self.nc.engines[eng].nop(nofuse=True, hint="wsplit")
                        nop.ins.sync_info = SI(on_wait=[w], on_update=[])
                        newl.append(nop.ins)
                newl.append(inst)
            ordered[bbname] = newl
        return _orig_lower(self, ordered)

    tile_mod.TileContext._lower_ordered_insts = _lower_with_wait_split
    _patched = True


def _fast_dispatch():
    """Enable concourse's C++ fast-path jit dispatch (drops the bass_exec
    effect token). Roughly halves the per-exec python dispatch overhead on
    the axon client; computation semantics are unchanged."""
    try:
        import jax
        import concourse.bass2jax  # noqa: F401  (registers the config)
        jax.config.update("bass_fast_dispatch", True)
    except Exception:
        pass


def _tr_set(SG, n_ship):
    """Positions of device-transposed supers: spread every 4th slot so the
    PE-heavy transpose work interleaves with DMA-heavy shipped supers."""
    n_tr = SG - n_ship
    slots = [i for i in range(SG) if i % 4 == 1]
    if len(slots) < n_tr:
        slots += [i for i in range(SG) if i % 4 == 3]
    return set(slots[:n_tr])


# consts16 column offsets (inside the last f16 data chunk)
C16_W1, C16_W2, C16_IOTA, C16_IDH, C16_ONE = 0, 512, 514, 642, 770
C16_END = 771
# consts32 column offsets
C32_WP, C32_BP, C32_B1, C32_EB, C32_IDF, C32_REL = 0, 512, 768, 770, 771, 899


def _build_nc(n_super_per_group):
    import concourse.bass as bass
    import concourse.mybir as mybir
    from concourse.tile import TileContext
    from collections import deque

    dt = mybir.dt
    f32 = dt.float32
    f16 = dt.float16
    Alu = mybir.AluOpType
    Act = mybir.ActivationFunctionType

    SG = n_super_per_group
    n_super = GROUPS * SG
    n_sub = n_super * SPS

    nc = bass.Bass(target_bir_lowering=False, use_seq_codegen=True)

    n_ship = (SG * SHIP_NUM + SHIP_DEN - 1) // SHIP_DEN
    n_xa = GROUPS * n_ship
    CH_C16 = n_super + n_xa

    # f16 blob: xT chunks | natural-x chunks (ones col interleaved) | consts
    data = nc.declare_dram_parameter("data", [CH_C16 + 1, SUB, WCH], f16, isOutput=False)
    c32in = nc.declare_dram_parameter("c32", [128, C32_REL + n_sub], f32, isOutput=False)
    out_sh = nc.declare_dram_parameter("out", [SEGS_PER_CORE, D], f32, isOutput=True)

    from contextlib import ExitStack
    with TileContext(nc) as tc:
        with ExitStack() as stk:
            ec = stk.enter_context
            cpool = ec(tc.tile_pool(name="consts", bufs=1))
            xapool = ec(tc.tile_pool(name="xa", bufs=18))
            xtpool = ec(tc.tile_pool(name="xt", bufs=20))
            sxtpool = ec(tc.tile_pool(name="sxt", bufs=10))
            thpool = ec(tc.tile_pool(name="th", bufs=10))
            e4pool = ec(tc.tile_pool(name="e4", bufs=6))
            oepool = ec(tc.tile_pool(name="oe", bufs=40))
            ufpool = ec(tc.tile_pool(name="uflush", bufs=2))
            sutpool = ec(tc.tile_pool(name="sut", bufs=2))
            rdpool = ec(tc.tile_pool(name="rd", bufs=2))
            osbpool = ec(tc.tile_pool(name="osb", bufs=2))
            # ---- constants into SBUF (2 DMAs)
            c16 = cpool.tile([128, C16_END], f16, tag="c16")
            nc.sync.dma_start(out=c16[:, :], in_=data[CH_C16, :, 0:C16_END])
            c32 = cpool.tile([128, C32_REL + n_sub], f32, tag="c32")
            nc.sync.dma_start(out=c32[:, :], in_=c32in[:, :])

            w1sb = c16[:, C16_W1:C16_W1 + 512]
            w2c = c16[:, C16_W2:C16_W2 + 2]
            iota = c16[:, C16_IOTA:C16_IOTA + 128]
            idh = c16[:, C16_IDH:C16_IDH + 128]
            onescol = c16[:, C16_ONE:C16_ONE + 1]
            wpsb = c32[:, C32_WP:C32_WP + 512]
            bpb = c32[:, C32_BP:C32_BP + 256]
            b1c = c32[:, C32_B1:C32_B1 + 2]
            ebias = c32[:, C32_EB:C32_EB + 1]
            idf = c32[:, C32_IDF:C32_IDF + 128]

            phpool = ec(tc.tile_pool(name="ph", bufs=3, space="PSUM"))
            pupool = ec(tc.tile_pool(name="pu", bufs=2, space="PSUM"))
            eppool = ec(tc.tile_pool(name="ep", bufs=1, space="PSUM"))
            pxtpool = ec(tc.tile_pool(name="pxt", bufs=1, space="PSUM"))
            ps4pool = ec(tc.tile_pool(name="ps4", bufs=1, space="PSUM"))

            SG_EFF = max(1, int(SG * WORK_FRAC))
            trs = _tr_set(SG, n_ship) if SG_EFF == SG else set()

            # deferred pooling matmuls: emitted one per MLP matmul of later
            # supers so each oe LDWEIGHTS hides under a 512-col MLP stream
            pq = deque()          # (group, emit_fn)
            pend_cnt = [0] * GROUPS
            group_done = [False] * GROUPS

            def emit_epilogue(g, pu):
                # out = (U @ Wp) / denom + bp  for this group's 128 segments
                uf = ufpool.tile([128, 257], f32, tag="uf")
                nc.vector.tensor_copy(out=uf[:, :], in_=pu[:, 0:257])
                ep = eppool.tile([128, 512], f32, tag="ep")
                put = ep[:, 0:256]
                nc.tensor.transpose(put[:, 0:128], uf[:, 0:128], idf)
                nc.tensor.transpose(put[:, 128:256], uf[:, 128:256], idf)
                sut = sutpool.tile([128, 256], f32, tag="sut")
                nc.vector.tensor_copy(out=sut[:, :], in_=put[:, :])
                po = ep[:, 256:512]
                nc.tensor.matmul(po[:, :], lhsT=sut[:, 0:128], rhs=wpsb[:, 0:256], start=True, stop=False)
                nc.tensor.matmul(po[:, :], lhsT=sut[:, 128:256], rhs=wpsb[:, 256:512], start=False, stop=True)
                rd = rdpool.tile([128, 1], f32, tag="rd")
                nc.vector.reciprocal(out=rd[:, :], in_=uf[:, 256:257])
                osb = osbpool.tile([128, 256], f32, tag="osb")
                nc.vector.scalar_tensor_tensor(
                    out=osb[:, :],
                    in0=po[:, :],
                    scalar=rd[:, 0:1],
                    in1=bpb[:, :],
                    op0=Alu.mult,
                    op1=Alu.add,
                )
                nc.sync.dma_start(
                    out=out_sh[g * 128 : (g + 1) * 128, :], in_=osb[:, :]
                )

            def pool_slot():
                if not pq:
                    return
                g2, fn = pq.popleft()
                fn()
                pend_cnt[g2] -= 1
                if pend_cnt[g2] == 0 and group_done[g2]:
                    emit_epilogue(g2, pu_of[g2])

            pu_of = {}
            for g in range(GROUPS):
                pu = pupool.tile([128, 257], f32, tag="pu")
                pu_of[g] = pu
                ps4b = ps4pool.tile([128, 16], f32, tag="ps4")
                rel_sb = c32[:, C32_REL + g * SG * SPS : C32_REL + (g + 1) * SG * SPS]
                xnats = [None] * 4
                xkinds = [None] * 4
                last_flushed = -1
                ship_i = 0
                for it in range(SG_EFF):
                    sidx = g * SG + it           # super-tile index
                    xt = xtpool.tile([128, 1024], f16, tag="xt")
                    nc.sync.dma_start(out=xt[:, :], in_=data[sidx, :, 0:1024])
                    if it not in trs:
                        xa = xapool.tile([128, WCH], f16, tag="xa")
                        nc.sync.dma_start(
                            out=xa[:, :], in_=data[n_super + g * n_ship + ship_i, :, :]
                        )
                        ship_i += 1
                        xnats[it % 4] = xa
                        xkinds[it % 4] = "xa"
                    else:
                        pxt = pxtpool.tile([128, 1024], f16, tag="pxt")
                        for j in range(SPS):
                            for kb in range(2):
                                nc.tensor.transpose(
                                    pxt[:, j * 256 + kb * 128 : j * 256 + (kb + 1) * 128],
                                    xt[:, kb * 512 + j * 128 : kb * 512 + (j + 1) * 128],
                                    idh,
                                )
                        sxt = sxtpool.tile([128, 1024], f16, tag="sxt")
                        nc.vector.tensor_copy(out=sxt[:, :], in_=pxt[:, :])
                        xnats[it % 4] = sxt
                        xkinds[it % 4] = "sxt"

                    # hT = W1^T x^T (2 dout blocks x 2 k blocks); one deferred
                    # pooling matmul rides after each MLP matmul
                    ph0 = phpool.tile([128, 512], f32, tag="ph")
                    ph1 = phpool.tile([128, 512], f32, tag="ph")
                    for dblk, ph in ((0, ph0), (1, ph1)):
                        for k in range(2):
                            nc.tensor.matmul(
                                ph[:, :],
                                lhsT=w1sb[:, (2 * k + dblk) * 128 : (2 * k + dblk + 1) * 128],
                                rhs=xt[:, k * 512 : (k + 1) * 512],
                                start=(k == 0),
                                stop=(k == 1),
                            )
                            pool_slot()
                    # tanh(h + b1)  (ACT, per-partition bias)
                    th0 = thpool.tile([128, 512], f16, tag="th0")
                    th1 = thpool.tile([128, 512], f16, tag="th1")
                    nc.scalar.activation(th0[:, :], ph0[:, :], Act.Tanh, bias=b1c[:, 0:1])
                    nc.scalar.activation(th1[:, :], ph1[:, :], Act.Tanh, bias=b1c[:, 1:2])

                    # s^T columns: ps4[node, j] = sum_dout th[dout, node] w2[dout]
                    # (own PSUM bank - a start=True matmul marks its whole 2KB
                    # zero-region pending, so it must not share a bank with the
                    # long-lived pu accumulator; 4 regions, one exp per 4 supers)
                    ps4 = ps4b[:, 4 * (it % 4) : 4 * (it % 4) + 4]
                    for j in range(SPS):
                        nc.tensor.matmul(
                            ps4[:, j : j + 1],
                            lhsT=th0[:, j * 128 : (j + 1) * 128],
                            rhs=w2c[:, 0:1],
                            start=True,
                            stop=False,
                            skip_group_check=True,
                        )
                        nc.tensor.matmul(
                            ps4[:, j : j + 1],
                            lhsT=th1[:, j * 128 : (j + 1) * 128],
                            rhs=w2c[:, 1:2],
                            start=False,
                            stop=True,
                            skip_group_check=True,
                        )

                    # e = exp(s + b2 - C), batched over up to 4 supers
                    if it % 4 == 3 or it == SG_EFF - 1:
                        b0 = last_flushed + 1
                        e4b = e4pool.tile([128, 4 * SPS], f32, tag="e4")
                        nc.scalar.activation(
                            e4b[:, 4 * (b0 % 4) : 4 * (it % 4) + 4],
                            ps4b[:, 4 * (b0 % 4) : 4 * (it % 4) + 4],
                            Act.Exp,
                            bias=ebias[:, 0:1],
                        )
                        pend = list(range(b0, it + 1))
                        last_flushed = it
                    else:
                        pend = None

                    # per subtile: Oe = (iota==rel) * e (DVE, emitted now);
                    # U += Oe^T @ [x|1] (PE, deferred into pool_slot()s)
                    if pend is None:
                        continue
                    for pit in pend:
                        xnat = xnats[pit % 4]
                        kind = xkinds[pit % 4]
                        relbase = pit * SPS
                        for j in range(SPS):
                            oe = oepool.tile([128, 128], f16, tag="oe")
                            nc.vector.tensor_scalar(
                                out=oe[:, :],
                                in0=iota[:, :],
                                scalar1=rel_sb[:, relbase + j : relbase + j + 1],
                                scalar2=e4b[:, 4 * (pit % 4) + j : 4 * (pit % 4) + j + 1],
                                op0=Alu.is_equal,
                                op1=Alu.mult,
                            )
                            first = pit == 0 and j == 0
                            last = pit == SG_EFF - 1 and j == SPS - 1

                            def mk(oe=oe, xnat=xnat, kind=kind, j=j,
                                   first=first, last=last, pu=pu):
                                def emit():
                                    if kind == "xa":
                                        nc.tensor.matmul(
                                            pu[:, 0:257],
                                            lhsT=oe[:, :],
                                            rhs=xnat[:, j * 257 : (j + 1) * 257],
                                            start=first,
                                            stop=last,
                                            skip_group_check=True,
                                        )
                                    else:
                                        nc.tensor.matmul(
                                            pu[:, 0:256],
                                            lhsT=oe[:, :],
                                            rhs=xnat[:, j * 256 : (j + 1) * 256],
                                            start=first,
                                            stop=last,
                                            skip_group_check=True,
                                        )
                                        nc.tensor.matmul(
                                            pu[:, 256:257],
                                            lhsT=oe[:, :],
                                            rhs=onescol[:, 0:1],
                                            start=False,
                                            stop=last,
                                            skip_group_check=True,
                                        )
                                return emit

                            pq.append((g, mk()))
                            pend_cnt[g] += 1
                group_done[g] = True
                if pend_cnt[g] == 0:
                    emit_epilogue(g, pu_of[g])
            # drain remaining deferred pooling matmuls
            while pq:
                pool_slot()
    return nc


def _prepare(x, batch, W1, b1, w2, b2, Wp, bp):
    _patch_drain()
    _fast_dispatch()

    x = np.asarray(x, dtype=np.float32)
    batch_np = np.asarray(batch).astype(np.int64)
    W1 = np.asarray(W1, dtype=np.float32)
    b1 = np.asarray(b1, dtype=np.float32)
    w2 = np.asarray(w2, dtype=np.float32)
    b2 = float(np.asarray(b2))
    Wp = np.asarray(Wp, dtype=np.float32)
    bp = np.asarray(bp, dtype=np.float32)

    n, d = x.shape
    assert (n, d) == (N, D)

    # piece p (p = 0..31): nodes whose segment is in [128p, 128(p+1))
    bounds = np.searchsorted(batch_np, np.arange(0, B + 1, CHUNK))  # [33]
    piece_nodes = np.diff(bounds)
    SG = int(np.ceil(piece_nodes.max() / (SPS * SUB)))
    n_super = GROUPS * SG
    n_sub = n_super * SPS

    n_ship = (SG * SHIP_NUM + SHIP_DEN - 1) // SHIP_DEN
    n_xa = GROUPS * n_ship
    CH_C16 = n_super + n_xa

    nc = _build_nc(SG)

    f16 = np.float16
    # ---- consts16 chunk (shared by all cores)
    c16 = np.zeros((SUB, WCH), dtype=f16)
    for k in range(2):
        for dblk in range(2):
            c16[:, C16_W1 + (2 * k + dblk) * 128 : C16_W1 + (2 * k + dblk + 1) * 128] = (
                W1[k * 128 : (k + 1) * 128, dblk * 128 : (dblk + 1) * 128]
            ).astype(f16)
    c16[:, C16_W2:C16_W2 + 2] = np.stack([w2[0:128], w2[128:256]], axis=1).astype(f16)
    c16[:, C16_IOTA:C16_IOTA + 128] = np.tile(
        np.arange(128, dtype=f16)[None, :], (128, 1)
    )
    c16[:, C16_IDH:C16_IDH + 128] = np.eye(128, dtype=f16)
    c16[:, C16_ONE:C16_ONE + 1] = 1.0

    # ---- consts32 (wpsb | bpb | b1c | ebias | idf | relT), rel is per-core
    c32_base = np.zeros((128, C32_REL + n_sub), dtype=np.float32)
    c32_base[:, C32_WP:C32_WP + 256] = Wp[0:128, :]
    c32_base[:, C32_WP + 256:C32_WP + 512] = Wp[128:256, :]
    c32_base[:, C32_BP:C32_BP + 256] = np.tile(bp[None, :], (128, 1))
    c32_base[:, C32_B1:C32_B1 + 2] = np.stack([b1[0:128], b1[128:256]], axis=1)
    c32_base[:, C32_EB] = b2 - C_OFF
    c32_base[:, C32_IDF:C32_IDF + 128] = np.eye(128, dtype=np.float32)

    x16 = x.astype(f16)

    in_maps = []
    trs = _tr_set(SG, n_ship)
    ship_order = [it for it in range(SG) if it not in trs]
    for c in range(NCORES):
        xflat = np.zeros((n_sub * SUB, D), dtype=f16)
        rel_c = np.full(n_sub * SUB, -1.0, dtype=np.float32)
        for g in range(GROUPS):
            p = c * GROUPS + g
            plo, phi = int(bounds[p]), int(bounds[p + 1])
            npc = phi - plo
            off = g * SG * SPS * SUB
            xflat[off : off + npc] = x16[plo:phi]
            rel_c[off : off + npc] = (batch_np[plo:phi] - (p * CHUNK)).astype(np.float32)

        data_c = np.zeros((CH_C16 + 1, SUB, WCH), dtype=f16)
        # xT chunks: row p, col k*512+n = x[node n, din 128k+p]
        data_c[:n_super, :, 0:1024] = (
            xflat.reshape(n_super, SPS * SUB, 2, 128)
            .transpose(0, 3, 2, 1)
            .reshape(n_super, 128, 2 * SPS * SUB)
        )
        # natural-x chunks for shipped supers, ones col interleaved
        xnat_all = xflat.reshape(n_super, SPS, SUB, D).transpose(0, 2, 1, 3)
        ship_idx = [g * SG + it for g in range(GROUPS) for it in ship_order]
        dview = data_c[n_super:CH_C16].reshape(n_xa, SUB, SPS, 257)
        dview[:, :, :, 0:256] = xnat_all[ship_idx]
        dview[:, :, :, 256] = 1.0
        data_c[CH_C16] = c16

        c32_c = c32_base.copy()
        c32_c[:, C32_REL:] = rel_c.reshape(n_sub, SUB).T  # [128, n_sub]

        in_maps.append({"data": data_c, "c32": c32_c})

    return nc, in_maps


def kernel(x, batch, W1, b1, w2, b2, Wp, bp):
    from concourse.bass_utils import run_bass_kernel_spmd

    nc, in_maps = _prepare(x, batch, W1, b1, w2, b2, Wp, bp)
    import kernel as _self
    res = run_bass_kernel_spmd(nc, in_maps, core_ids=list(range(NCORES)))
    _self._last_res = res
    out = np.concatenate([res.results[c]["out"] for c in range(NCORES)], axis=0)
    return out.astype(np.float32)
